# revision 11
# baseline (speedup 1.0000x reference)
"""Trainium2 Bass kernel for masked graph-convolution interaction.

Math (reference):
    wf = node_features @ weight                              # [N, D]
    T[i,d,j] = wf[i,d] * wf[j,d] * mh[i,j]
    S[a,d,j] = sum_i adj[a,i] * T[i,d,j]
    out[a,d] = sum_j S[a,d,j] * mf[a,j] / ncnt[a]^2

Reformulated per output row a:
    X_a[i,d] = adj[a,i] * wf[i,d]
    Y_a[j,d] = sum_i mh[i,j] * X_a[i,d]        (PE matmul, mh tiles as weights)
    Z_a[j,d] = Y_a[j,d] * wf[j,d]              (DVE elementwise)
    out[a,d] = sum_j mfT[j,a] * Z_a[j,d]       (PE matvec, Z as weights ->
                                                column a of outT [d, a] PSUM)
    out[a,:] *= 1 / ncnt[a]^2                  (after PE transpose of outT)

Sharding: row-split of a across 8 cores (128 rows each); mh / wf replicated.
"""

import numpy as np

N = 1024
DIN = 256
DOUT = 128
NCORES = 8
ROWS = N // NCORES  # 128 output rows per core
P = 128

# "float32" (exact) or "bfloat16" (2x faster PE phase A/B)
_DTYPE = "float32"

_CACHE = {}


def _np_dt(name):
    if name == "float32":
        return np.float32
    import ml_dtypes

    return ml_dtypes.bfloat16


def _build(dtype_name):
    """Build + compile the Bass module (shared across all 8 cores, SPMD)."""
    import concourse.bass as bass
    import concourse.tile as tile
    from concourse import bacc, mybir
    from concourse._compat import axon_active
    from concourse.masks import make_identity

    dt = mybir.dt.float32 if dtype_name == "float32" else mybir.dt.bfloat16
    f32 = mybir.dt.float32
    Copy = mybir.ActivationFunctionType.Copy

    nc = bacc.Bacc(
        "TRN2",
        target_bir_lowering=False,
        debug=not axon_active(),
        num_devices=NCORES,
    )

    mh_d = nc.dram_tensor("mh", [N, N], dt, kind="ExternalInput").ap()
    adjT_d = nc.dram_tensor("adjT", [N, ROWS], f32, kind="ExternalInput").ap()
    mfT_d = nc.dram_tensor("mfT", [N, ROWS], dt, kind="ExternalInput").ap()
    nfT_d = nc.dram_tensor("nfT", [DIN, N], f32, kind="ExternalInput").ap()
    w_d = nc.dram_tensor("w", [DIN, DOUT], f32, kind="ExternalInput").ap()
    ncnt_d = nc.dram_tensor("ncnt", [ROWS, 1], f32, kind="ExternalInput").ap()
    out_d = nc.dram_tensor("out", [ROWS, DOUT], f32, kind="ExternalOutput").ap()

    IC = N // P  # 8 contraction chunks over i
    JC = N // P  # 8 chunks over j
    KC = DIN // P  # 2 chunks over k (wf compute)
    G4 = 4  # rows per group (psum free dim 4*128 = 512)
    NG = ROWS // G4  # 32 groups per core

    with tile.TileContext(nc) as tc:
        with (
            tc.tile_pool(name="const", bufs=1) as cpool,
            tc.tile_pool(name="x", bufs=3) as xpool,
            tc.tile_pool(name="z", bufs=10) as zpool,
            tc.tile_pool(name="py", bufs=4, space="PSUM") as pypool,
            tc.tile_pool(name="pout", bufs=1, space="PSUM") as popool,
        ):
            # ---- resident tiles + input DMA ----
            mh_sb = cpool.tile([P, IC * N], dt, tag="mh")
            for ic in range(IC):
                nc.sync.dma_start(
                    mh_sb[:, ic * N : (ic + 1) * N], mh_d[ic * P : (ic + 1) * P, :]
                )
            adjT_sb = cpool.tile([P, N], f32, tag="adjT")
            mfT_sb = cpool.tile([P, N], dt, tag="mfT")
            for c in range(N // P):
                nc.sync.dma_start(
                    adjT_sb[:, c * P : (c + 1) * P], adjT_d[c * P : (c + 1) * P, :]
                )
                nc.sync.dma_start(
                    mfT_sb[:, c * P : (c + 1) * P], mfT_d[c * P : (c + 1) * P, :]
                )
            nfT_sb = cpool.tile([P, KC * N], f32, tag="nfT")
            for kc in range(KC):
                for c in range(N // P):
                    nc.sync.dma_start(
                        nfT_sb[:, (kc * (N // P) + c) * P : (kc * (N // P) + c + 1) * P],
                        nfT_d[kc * P : (kc + 1) * P, c * P : (c + 1) * P],
                    )
            w_sb = cpool.tile([P, KC * DOUT], f32, tag="w")
            for kc in range(KC):
                nc.sync.dma_start(
                    w_sb[:, kc * DOUT : (kc + 1) * DOUT],
                    w_d[kc * P : (kc + 1) * P, :],
                )
            ncnt_sb = cpool.tile([P, 1], f32, tag="ncnt")
            nc.sync.dma_start(ncnt_sb[:], ncnt_d[:])

            # ---- setup compute ----
            # wf[n,d] = sum_k nf[n,k] w[k,d]; chunks of 128 n-rows
            wf_sb = cpool.tile([P, N], f32, tag="wf")
            for c in range(N // P):
                pt = pypool.tile([P, 512], f32, tag="py")
                for kc in range(KC):
                    nc.tensor.matmul(
                        pt[:, :DOUT],
                        lhsT=nfT_sb[:, (kc * (N // P) + c) * P : (kc * (N // P) + c + 1) * P],
                        rhs=w_sb[:, kc * DOUT : (kc + 1) * DOUT],
                        start=(kc == 0),
                        stop=(kc == KC - 1),
                    )
                nc.vector.tensor_copy(wf_sb[:, c * DOUT : (c + 1) * DOUT], pt[:, :DOUT])

            # wf4: wf[jc] replicated 4x along free dim, for Z = Y * wf
            wf4_sb = cpool.tile([P, JC * 512], f32, tag="wf4")
            for jc in range(JC):
                for r in range(G4):
                    nc.vector.tensor_copy(
                        wf4_sb[:, jc * 512 + r * DOUT : jc * 512 + (r + 1) * DOUT],
                        wf_sb[:, jc * DOUT : (jc + 1) * DOUT],
                    )

            # inv_nc2 = 1 / ncnt^2
            sq_sb = cpool.tile([P, 1], f32, tag="sq")
            inv_sb = cpool.tile([P, 1], f32, tag="inv")
            nc.vector.tensor_mul(sq_sb[:], ncnt_sb[:], ncnt_sb[:])
            nc.vector.reciprocal(inv_sb[:], sq_sb[:])

            id_sb = cpool.tile([P, P], f32, tag="ident")
            make_identity(nc, id_sb[:])

            # outT[d, (s,b)]: accumulated over jc by the per-a matvecs.
            # One PSUM bank per s-class: concurrent start=True matmuls into
            # the same bank within the PE drain window lose all but the last
            # result, so the 4 rows of a group must land in 4 distinct banks.
            outT_s = [
                popool.tile([P, NG], f32, tag=f"outT{s}", name=f"outT{s}")
                for s in range(G4)
            ]

            # ---- main loop: 32 groups of 4 rows ----
            for b in range(NG):
                # X[(ic), s]: X_a[i,d] = adj[a,i] * wf[i,d]  (ACT copy w/ scale)
                x_t = xpool.tile([P, IC * 512], dt, tag="X")
                for ic in range(IC):
                    for s in range(G4):
                        a = b * G4 + s
                        dst = x_t[:, ic * 512 + s * DOUT : ic * 512 + (s + 1) * DOUT]
                        src = wf_sb[:, ic * DOUT : (ic + 1) * DOUT]
                        sc = adjT_sb[:, ic * P + a : ic * P + a + 1]
                        # split across ACT and DVE so neither engine paces
                        # the batch loop (PE should be the only limiter)
                        if s % 2 == 0:
                            nc.scalar.activation(dst, src, Copy, scale=sc)
                        else:
                            nc.vector.tensor_scalar_mul(dst, src, sc)
                z_ts = []
                for jc in range(JC):
                    py = pypool.tile([P, 512], f32, tag="py")
                    for ic in range(IC):
                        nc.tensor.matmul(
                            py[:],
                            lhsT=mh_sb[:, ic * N + jc * P : ic * N + (jc + 1) * P],
                            rhs=x_t[:, ic * 512 : (ic + 1) * 512],
                            start=(ic == 0),
                            stop=(ic == IC - 1),
                        )
                    z_t = zpool.tile([P, 512], dt, tag="Z")
                    nc.vector.tensor_mul(
                        z_t[:], py[:], wf4_sb[:, jc * 512 : (jc + 1) * 512]
                    )
                    z_ts.append(z_t)
                # matvecs trail the whole batch: by the time the PE reaches
                # them, every Z is ready -> no DVE-wait bubbles in the stream
                for jc in range(JC):
                    for s in range(G4):
                        a = b * G4 + s
                        nc.tensor.matmul(
                            outT_s[s][:, b : b + 1],
                            lhsT=z_ts[jc][:, s * DOUT : (s + 1) * DOUT],
                            rhs=mfT_sb[:, jc * P + a : jc * P + a + 1],
                            start=(jc == 0),
                            stop=(jc == JC - 1),
                            skip_group_check=True,
                        )

            # ---- finish: transpose outT -> [(s,b), d], scale, store ----
            # row r = s*NG + b of the transpose corresponds to out row 4b+s;
            # ncnt comes host-permuted to match, DMA de-permutes at the end.
            outT_sb = cpool.tile([P, ROWS], f32, tag="outT_sb")
            for s in range(G4):
                nc.vector.tensor_copy(
                    outT_sb[:, s * NG : (s + 1) * NG], outT_s[s][:]
                )
            tr_ps = pypool.tile([P, 512], f32, tag="py")
            nc.tensor.transpose(tr_ps[:, :P], outT_sb[:], id_sb[:])
            out_sb = cpool.tile([ROWS, DOUT], f32, tag="out_sb")
            nc.vector.tensor_scalar_mul(out_sb[:], tr_ps[:, :DOUT], inv_sb[:])
            for s in range(G4):
                nc.sync.dma_start(
                    out_d[s :: G4, :], out_sb[s * NG : (s + 1) * NG, :]
                )

    nc.compile()
    return nc


def _prep_inputs(inputs, dtype_name):
    """Host-side sharding + layout prep. Returns per-core input maps."""
    npdt = _np_dt(dtype_name)
    nf = np.asarray(inputs["node_features"], dtype=np.float32)
    adj = np.asarray(inputs["adjacency_matrix"], dtype=np.float32)
    mf = np.asarray(inputs["mask_father"], dtype=np.float32)[:, 0, :]
    ncnt = np.asarray(inputs["neighbor_count"], dtype=np.float32)
    mh = np.asarray(inputs["mask_hadamard"], dtype=np.float32)[:, 0, :]
    w = np.asarray(inputs["weight"], dtype=np.float32)

    mh_x = np.ascontiguousarray(mh).astype(npdt)
    nfT = np.ascontiguousarray(nf.T)
    in_maps = []
    for c in range(NCORES):
        rows = slice(c * ROWS, (c + 1) * ROWS)
        in_maps.append(
            {
                "mh": mh_x,
                "adjT": np.ascontiguousarray(adj[rows].T),
                "mfT": np.ascontiguousarray(mf[rows].T).astype(npdt),
                "nfT": nfT,
                "w": w,
                # permuted to (s, b) order: row s*32+b holds ncnt[4b+s]
                "ncnt": np.ascontiguousarray(
                    ncnt[rows].reshape(ROWS // 4, 4).T.reshape(ROWS, 1)
                ),
            }
        )
    return in_maps


def _run(inputs, trace=False):
    from concourse import bass_utils

    key = _DTYPE
    if key not in _CACHE:
        _CACHE[key] = _build(key)
    nc = _CACHE[key]
    in_maps = _prep_inputs(inputs, key)
    res = bass_utils.run_bass_kernel_spmd(
        nc, in_maps, core_ids=list(range(NCORES)), trace=trace
    )
    out = np.concatenate([r["out"] for r in res.results], axis=0)
    return out, res


def kernel(**inputs):
    out, _ = _run(inputs, trace=False)
    return out
